# revision 3
# baseline (speedup 1.0000x reference)
"""Causal GQA self-attention (B=2, T=2048, C=1024, 16 q-heads / 4 kv-heads,
RoPE, causal softmax, output projection) on 8 Trainium2 NeuronCores.

Sharding: core c = b*4 + hg handles batch b (2-way data parallel) and
head-group hg (4-way tensor parallel: its 4 q-heads + their shared kv head).
W_qkv is column-sharded, W_proj row-sharded; each core emits a partial
projection [2048, 1024] and the host sums the 4 partials per batch.

Device pipeline per core (bf16 matmul inputs, fp32 PSUM accumulate):
  1. qkv = x @ W_qkv_shard -> token-major tiles q01 [128,4t,256] (4 q heads),
     kt4 [128,4t,128] (k | k-dup), vone [128,4t,65] (ones | v)
  2. RoPE in token-major (head-dim pairs pre-permuted even|odd on the host,
     rotation = contiguous 32-blocks); k roped once then duplicated
  3. TWO multi-tile DMA-xbar transposes per 512-token chunk (block-fold 3D
     output): q01 -> qt [128,(4t,2dh),128] (all 4 heads at once),
     kt4 -> ktT [128,4t,128] (rows 0-63 kT, 64-127 kT-dup)
  4. flash-style per 512-wide q chunk: S^T = k @ qT (K=64), exp on ScalarE
     (1/sqrt(64) folded into the activation scale), causal masking only on
     diagonal 128-blocks, y^T[1+64, q] += [1|v]^T @ P^T (v_aug stationary)
  5. y^T row 0 is the softmax denominator: reciprocal_approx straight off
     PSUM + GPSIMD partition_broadcast + fused normalize-evacuate multiply
  6. out = yT.T @ W_proj_shard
"""

import sys

if "/opt/trn_rl_repo" not in sys.path:
    sys.path.insert(0, "/opt/trn_rl_repo")

import numpy as np
import ml_dtypes

B, T, C = 2, 2048, 1024
NH, NKV, HD = 16, 4, 64
THETA = 10000.0
NQ = NH // NKV          # q heads per core = 4
TT = T // 128           # 16 token tiles
NCHUNK = T // 512       # 4 q-chunks
BF16 = ml_dtypes.bfloat16

_CACHE = {}


def _build():
    """Build the SPMD Bass program (identical on all 8 cores)."""
    import concourse.mybir as mybir
    import concourse.tile as tile
    from concourse import bacc
    from concourse.bass import ts
    from contextlib import ExitStack

    dt = mybir.dt
    AF = mybir.ActivationFunctionType

    nc = bacc.Bacc("TRN2", target_bir_lowering=False, debug=False, num_devices=8)

    # host pre-shuffled, partition-major inputs (contiguous per partition)
    xt_d = nc.declare_dram_parameter("xT", [128, 8 * T], dt.bfloat16, isOutput=False)
    w_d = nc.declare_dram_parameter("w384", [128, 8 * 384], dt.bfloat16, isOutput=False)
    wo_d = nc.declare_dram_parameter("wo", [128, 2 * C], dt.bfloat16, isOutput=False)
    cs_d = nc.declare_dram_parameter(
        "cs", [128, NCHUNK * 4 * 128], dt.bfloat16, isOutput=False
    )
    out_d = nc.declare_dram_parameter("out", [T, C], dt.bfloat16, isOutput=True)

    with tile.TileContext(nc) as tc, ExitStack() as ctx:
        persist = ctx.enter_context(tc.tile_pool(name="persist", bufs=1))
        rope_tmp = ctx.enter_context(tc.tile_pool(name="rope_tmp", bufs=2))
        p_pool = ctx.enter_context(tc.tile_pool(name="p_pool", bufs=14))
        po_pool = ctx.enter_context(tc.tile_pool(name="po", bufs=6))
        bc_pool = ctx.enter_context(tc.tile_pool(name="bc", bufs=2))
        den_pool = ctx.enter_context(tc.tile_pool(name="den", bufs=2))
        yst_pool = ctx.enter_context(tc.tile_pool(name="yst", bufs=6))
        s_ps_pool = ctx.enter_context(tc.tile_pool(name="s_ps", bufs=3, space="PSUM"))
        pr_ps_pool = ctx.enter_context(tc.tile_pool(name="pr_ps", bufs=1, space="PSUM"))
        y_ps_pool = ctx.enter_context(tc.tile_pool(name="y_ps", bufs=1, space="PSUM"))

        # ---- persistent SBUF ----
        w_sb = persist.tile([128, 8, 384], dt.bfloat16)
        nc.sync.dma_start(w_sb[:], w_d.ap().rearrange("p (c n) -> p c n", c=8))
        xt_sb = []
        cs_sb = []
        for jc in range(NCHUNK):
            xt = persist.tile([128, 8, 512], dt.bfloat16, name=f"xtc{jc}")
            nc.sync.dma_start(
                xt[:],
                xt_d.ap()[:, ts(jc, 8 * 512)].rearrange("p (c t) -> p c t", c=8),
            )
            xt_sb.append(xt)
            # per chunk [128, 4t, 128]: cols 0:64 = cos x2, 64:128 = sin x2
            cst = persist.tile([128, 4, 128], dt.bfloat16, name=f"cs{jc}")
            nc.sync.dma_start(
                cst[:],
                cs_d.ap()[:, ts(jc, 4 * 128)].rearrange("p (n d) -> p n d", n=4),
            )
            cs_sb.append(cst)
        wo_sb = persist.tile([128, 2, C], dt.bfloat16)
        nc.sync.dma_start(wo_sb[:], wo_d.ap().rearrange("p (c n) -> p c n", c=2))

        vone = []   # per chunk: [128, 4, 65] = ones | v
        qt_sb = []  # per chunk: [128, 4t, 2dh, 128tok]; head = 2*dh + part//64
        kt_sb = []  # per chunk: [128, 4t, 128]: rows 0-63 kT, 64-127 kT dup
        ynt = [[None] * NCHUNK for _ in range(2)]     # [dimtile][chunk] [128,512]
        for d in range(2):
            for j in range(NCHUNK):
                ynt[d][j] = persist.tile([128, 512], dt.bfloat16, name=f"ynt{d}_{j}")

        # ---- phase 1+2: qkv, rope, transposes (per 4-token-tile chunk) ----
        for jc in range(NCHUNK):
            q01 = persist.tile([128, 4, 256], dt.bfloat16, name=f"q01_{jc}")
            kt4 = persist.tile([128, 4, 128], dt.bfloat16, name=f"kt4_{jc}")
            vo = persist.tile([128, 4, 65], dt.bfloat16, name=f"vone{jc}")
            vone.append(vo)
            for t4 in range(4):
                if jc == 0:
                    ps = y_ps_pool.tile(
                        [128, 384], dt.float32, tag=f"y{t4 % 2}", name="qkv0_ps"
                    )
                else:
                    ps = pr_ps_pool.tile(
                        [128, 384], dt.float32, tag="t", name="qkv_ps"
                    )
                for c in range(8):
                    nc.tensor.matmul(
                        ps[:],
                        lhsT=xt_sb[jc][:, c, ts(t4, 128)],
                        rhs=w_sb[:, c, :],
                        start=(c == 0),
                        stop=(c == 7),
                    )
                nc.vector.tensor_copy(q01[:, t4, :], ps[:, 0:256])
                nc.scalar.copy(kt4[:, t4, 0:64], ps[:, 256:320])
                nc.scalar.copy(vo[:, t4, 1:65], ps[:, 320:384])
            nc.gpsimd.memset(vo[:, :, 0:1], 1.0)

            # RoPE in place; tables tiled x2 heads on host, loop head-pairs
            csc = cs_sb[jc][:]
            qcos = csc[:, :, 0:64].rearrange("p f (h d) -> p f h d", h=2)
            qsin = csc[:, :, 64:128].rearrange("p f (h d) -> p f h d", h=2)
            qv = q01[:].rearrange("p f (h d) -> p f h d", h=4)
            t1 = rope_tmp.tile([128, 4, 4, 32], dt.bfloat16, tag="t1")
            t2 = rope_tmp.tile([128, 4, 4, 32], dt.bfloat16, tag="t2")
            t3 = rope_tmp.tile([128, 4, 4, 32], dt.bfloat16, tag="t3")
            t4_ = rope_tmp.tile([128, 4, 4, 32], dt.bfloat16, tag="t4")
            for hh in range(2):
                x1 = qv[:, :, ts(hh, 2), 0:32]
                x2 = qv[:, :, ts(hh, 2), 32:64]
                a1 = t1[:, :, ts(hh, 2), :]
                a2 = t2[:, :, ts(hh, 2), :]
                a3 = t3[:, :, ts(hh, 2), :]
                a4 = t4_[:, :, ts(hh, 2), :]
                nc.vector.tensor_mul(a1, x1, qcos)
                nc.vector.tensor_mul(a2, x2, qsin)
                nc.vector.tensor_mul(a3, x1, qsin)
                nc.vector.tensor_mul(a4, x2, qcos)
                nc.vector.tensor_sub(x1, a1, a2)
                nc.vector.tensor_add(x2, a3, a4)
            # k rope (one copy), then duplicate into cols 64:128
            kx1 = kt4[:, :, 0:32]
            kx2 = kt4[:, :, 32:64]
            kcos = csc[:, :, 0:32]
            ksin = csc[:, :, 64:96]
            k1 = rope_tmp.tile([128, 4, 32], dt.bfloat16, tag="k1")
            k2 = rope_tmp.tile([128, 4, 32], dt.bfloat16, tag="k2")
            k3 = rope_tmp.tile([128, 4, 32], dt.bfloat16, tag="k3")
            k4 = rope_tmp.tile([128, 4, 32], dt.bfloat16, tag="k4")
            nc.vector.tensor_mul(k1[:], kx1, kcos)
            nc.vector.tensor_mul(k2[:], kx2, ksin)
            nc.vector.tensor_mul(k3[:], kx1, ksin)
            nc.vector.tensor_mul(k4[:], kx2, kcos)
            nc.vector.tensor_sub(kx1, k1[:], k2[:])
            nc.vector.tensor_add(kx2, k3[:], k4[:])
            nc.vector.tensor_copy(kt4[:, :, 64:128], kt4[:, :, 0:64])

            # two multi-tile xbar transposes cover the whole chunk
            qt = persist.tile([128, 4, 2, 128], dt.bfloat16, name=f"qt{jc}")
            nc.sync.dma_start_transpose(qt[:], q01[:].rearrange("p a b -> p (a b)"))
            ktT = persist.tile([128, 4, 128], dt.bfloat16, name=f"ktT{jc}")
            nc.sync.dma_start_transpose(ktT[:], kt4[:].rearrange("p a b -> p (a b)"))
            qt_sb.append(qt)
            kt_sb.append(ktT)

        # ---- phase 3+4: attention + projection per 512-wide q chunk ----
        # y^T[65, q] = [1|v]^T @ P^T over k tiles; row 0 = softmax denominator.
        for j in range(NCHUNK):
            for hp in range(2):
                y_ps = y_ps_pool.tile(
                    [65, 2, 512], dt.float32, tag=f"y{hp}", name=f"y_ps{hp}"
                )
                for i in range(4 * j + 4):  # k tiles
                    ic, i4 = divmod(i, 4)
                    off = max(0, 128 * i - 512 * j)  # causal: valid q >= 128*i
                    for u in range(2):  # head 2hp+u; kT copy at partitions 64u
                        s_ps = s_ps_pool.tile(
                            [128, 512], dt.float32, tag="s", name="s_ps"
                        )
                        nc.tensor.matmul(
                            s_ps[:, off:512],
                            lhsT=kt_sb[ic][ts(u, 64), i4, :],
                            rhs=qt_sb[j][ts(u, 64), off // 128 : 4, hp, :],
                            start=True,
                            stop=True,
                        )
                        p_t = p_pool.tile([128, 512], dt.bfloat16, name="p_t")
                        nc.scalar.activation(
                            p_t[:, off:512], s_ps[:, off:512], AF.Exp, scale=0.125
                        )
                        if 128 * i >= 512 * j:  # diagonal block: causal mask
                            # keep where q_local - k_local >= 0, else 0
                            nc.gpsimd.affine_select(
                                p_t[:, off : off + 128],
                                p_t[:, off : off + 128],
                                pattern=[[1, 128]],
                                compare_op=mybir.AluOpType.is_ge,
                                fill=0.0,
                                base=0,
                                channel_multiplier=-1,
                            )
                        nc.tensor.matmul(
                            y_ps[:, u, off:512],
                            lhsT=vone[ic][:, i4, :],
                            rhs=p_t[:, off:512],
                            start=(i == 0),
                            stop=(i == 4 * j + 3),
                        )
                # den row -> reciprocal -> broadcast -> normalize off PSUM
                den = den_pool.tile([1, 2, 512], dt.float32)
                nc.vector.reciprocal_approx_fast(den[:], y_ps[0:1, :, :])
                bc = bc_pool.tile([65, 2, 512], dt.float32)
                nc.gpsimd.partition_broadcast(bc[:], den[:], channels=65)
                for u in range(2):
                    yst = yst_pool.tile([65, 512], dt.bfloat16)
                    nc.vector.tensor_mul(yst[:, :], y_ps[:, u, :], bc[:, u, :])
                    nc.scalar.dma_start(ynt[hp][j][ts(u, 64), :], yst[1:65, :])
            # projection for this chunk's 4 token tiles
            for t4 in range(4):
                tt = 4 * j + t4
                po = po_pool.tile([128, 1024], dt.bfloat16)
                for nn2 in range(2):
                    if j == NCHUNK - 1:
                        ps = s_ps_pool.tile(
                            [128, 512], dt.float32, tag="s", name="pr3s_ps"
                        )
                    else:
                        ps = pr_ps_pool.tile(
                            [128, 512], dt.float32, tag="t", name="pr_ps"
                        )
                    for dtile in range(2):
                        nc.tensor.matmul(
                            ps[:],
                            lhsT=ynt[dtile][j][:, ts(t4, 128)],
                            rhs=wo_sb[:, dtile, ts(nn2, 512)],
                            start=(dtile == 0),
                            stop=(dtile == 1),
                        )
                    if j == NCHUNK - 1:
                        nc.scalar.copy(po[:, ts(nn2, 512)], ps[:])
                    else:
                        nc.vector.tensor_copy(po[:, ts(nn2, 512)], ps[:])
                nc.scalar.dma_start(out_d.ap()[ts(tt, 128), :], po[:])

    nc.finalize()
    return nc


def _host_inputs(x, W_qkv, W_proj):
    """Per-core input maps (host-side sharding + partition-major layout)."""
    perm = np.concatenate([np.arange(0, HD, 2), np.arange(1, HD, 2)])  # even|odd
    inv = 1.0 / THETA ** (np.arange(0, HD, 2, dtype=np.float64) / HD)  # [32]
    ang = np.arange(T, dtype=np.float64)[:, None] * inv[None, :]       # [T, 32]
    cos2 = np.tile(np.cos(ang), (1, 2))
    sin2 = np.tile(np.sin(ang), (1, 2))
    cs = np.concatenate([cos2, sin2], axis=1).astype(BF16)             # [T, 128]
    # [T, 128] -> [128, TT*128] partition-major (token t = n*128 + p)
    cs_pm = np.ascontiguousarray(
        cs.reshape(TT, 128, 128).transpose(1, 0, 2).reshape(128, TT * 128)
    )

    def part_major(a, p=128):  # [R, cols] with R = n*p -> [p, n*cols]
        R, cols = a.shape
        n = R // p
        return np.ascontiguousarray(
            a.reshape(n, p, cols).transpose(1, 0, 2).reshape(p, n * cols)
        )

    in_maps = []
    for core in range(8):
        b, hg = divmod(core, 4)
        xT = x[b].T.astype(BF16)                                       # [C, T]
        # [C, T] -> [128, jc*(8*512)]: (c p), (jc t') -> p, jc, c, t'
        xt_pm = np.ascontiguousarray(
            xT.reshape(8, 128, NCHUNK, 512)
            .transpose(1, 2, 0, 3)
            .reshape(128, NCHUNK * 8 * 512)
        )
        cols = []
        for h in range(hg * NQ, hg * NQ + NQ):
            cols.append(W_qkv[:, h * HD : (h + 1) * HD][:, perm])
        kblk = W_qkv[:, NH * HD + hg * HD : NH * HD + (hg + 1) * HD][:, perm]
        vblk = W_qkv[:, (NH + NKV) * HD + hg * HD : (NH + NKV) * HD + (hg + 1) * HD]
        w384 = np.concatenate(cols + [kblk, vblk], axis=1).astype(BF16)
        wo = W_proj[hg * NQ * HD : (hg + 1) * NQ * HD, :].astype(BF16)
        in_maps.append(
            {
                "xT": xt_pm,
                "w384": part_major(w384),
                "wo": part_major(wo),
                "cs": cs_pm,
            }
        )
    return in_maps


def _run(in_maps):
    from concourse.bass_utils import run_bass_kernel_spmd

    if "nc" not in _CACHE:
        _CACHE["nc"] = _build()
    return run_bass_kernel_spmd(_CACHE["nc"], in_maps, core_ids=list(range(8)))


def kernel(x, W_qkv, W_proj):
    x = np.asarray(x, dtype=np.float32)
    W_qkv = np.asarray(W_qkv, dtype=np.float32)
    W_proj = np.asarray(W_proj, dtype=np.float32)
    res = _run(_host_inputs(x, W_qkv, W_proj))
    out = np.zeros((B, T, C), dtype=np.float32)
    for core in range(8):
        b = core // 4
        out[b] += res.results[core]["out"].astype(np.float32)
    return out
